# revision 32
# baseline (speedup 1.0000x reference)
"""Bass/Trainium2 kernel for nn_MultiHeadedAttention (GQA + RoPE + causal attention).

Sharding: 8 cores = 2 batch groups x 4 head-groups.
Core c: batch b=c//4, head group j=c%4 (q heads 4j..4j+3, kv head j).
Output projection is column-sharded after per-token-block AllGathers of
ctx^T; host concatenates the disjoint output slices.

Single interleaved pipeline. Rounds t=0..3 each do
  proj(t) -> attn(qb=t) -> gather(qb=t), with outproj(qb) matmuls woven
into later attention rounds as PE filler once their gathers have landed.
Scores matmuls run 2 deep ahead of the attention*V matmuls so the exp
(Scalar engine) latency never stalls the PE. The softmax denominator is
built by pair-summing exp tiles on DVE and accumulating a ones[128x128]
matmul per pair into PSUM (already broadcast across partitions).
Diagonal (causal-edge) tiles are shrunk to their live q range. A dummy
AllGather at program start absorbs the CC-channel init + cross-core
launch skew during the initial weight DMAs.
Compute is bf16 with fp32 PSUM accumulation.
"""

import os
import sys

sys.path.insert(0, "/opt/trn_rl_repo")
import numpy as np


B, S, HID = 2, 2048, 2048
NH, NKV, D = 16, 4, 128
N_CORES = 8
GROUPS = [[0, 1, 2, 3], [4, 5, 6, 7]]
HLOC = 4          # q heads per core
TB = 512          # token block (matmul moving dim)
NTB = S // TB     # 4
HT = HID // 128   # 16 hid tiles
SCALE = float(D) ** -0.5

LAST_RESULTS = None  # stash for test harness timing


def _analyze_mask(mask):
    """Per (qblock, ktile): live tiles and mixed-mask info (deduped).

    mixd[(qb, kt)] is None for fully-live tiles, else (uid, qlo, mlo, mhi):
    q columns < qlo are fully masked (skipped entirely), and the pattern
    uid must be multiplied over local q columns [mlo, mhi).
    """
    maskb = np.asarray(mask).astype(bool)
    live = []
    mixd = {}
    uniq = []
    keys = {}
    for qb in range(NTB):
        lv = []
        for kt in range(S // 128):
            sub = maskb[qb * TB:(qb + 1) * TB, kt * 128:(kt + 1) * 128]
            if not sub.any():
                continue
            lv.append(kt)
            if sub.all():
                mixd[(qb, kt)] = None
            else:
                rows_any = sub.any(axis=1)
                qlo = int(np.argmax(rows_any))
                rows_all = sub.all(axis=1)
                not_all = ~rows_all
                not_all[:qlo] = False
                idx = np.nonzero(not_all)[0]
                mlo, mhi = int(idx[0]), int(idx[-1]) + 1
                tile = np.ascontiguousarray(
                    sub[mlo:mhi, :].T.astype(np.float32))
                kb = (tile.tobytes(), mhi - mlo)
                if kb not in keys:
                    keys[kb] = len(uniq)
                    uniq.append(tile)
                mixd[(qb, kt)] = (keys[kb], qlo, mlo, mhi)
        live.append(lv)
    return live, mixd, uniq


def _build_program(live, mixd, n_u):
    import concourse.bass as bass  # noqa: F401
    import concourse.mybir as mybir
    from concourse import bacc, tile

    f32 = mybir.dt.float32
    bf16 = mybir.dt.bfloat16
    EXP = mybir.ActivationFunctionType.Exp

    nc = bacc.Bacc("TRN2", target_bir_lowering=False, debug=False,
                   num_devices=N_CORES)

    xT = nc.dram_tensor("xT", [HID, S], bf16, kind="ExternalInput")
    wq = nc.dram_tensor("wq", [HID, HLOC * D], bf16, kind="ExternalInput")
    wk = nc.dram_tensor("wk", [HID, D], bf16, kind="ExternalInput")
    wv = nc.dram_tensor("wv", [HID, D], bf16, kind="ExternalInput")
    wo = nc.dram_tensor("wo", [HID, TB], bf16, kind="ExternalInput")
    cosE = nc.dram_tensor("cosE", [D, S], bf16, kind="ExternalInput")
    sinP = nc.dram_tensor("sinP", [D, S], bf16, kind="ExternalInput")
    pswap = nc.dram_tensor("pswap", [128, 128], bf16, kind="ExternalInput")
    ident = nc.dram_tensor("ident", [128, 128], bf16, kind="ExternalInput")
    ones_in = nc.dram_tensor("ones_in", [128, 128], bf16, kind="ExternalInput")
    mmask = nc.dram_tensor("mmask", [max(n_u, 1) * 128, TB], bf16,
                           kind="ExternalInput")
    out_o = nc.dram_tensor("o", [S, TB], f32, kind="ExternalOutput")

    mm = nc.tensor.matmul

    with tile.TileContext(nc, num_cores=N_CORES) as tc:
        stk0 = nc.allow_low_precision("bf16 kernel; fp32 PSUM accumulate")
        stk0.__enter__()
        with (
            tc.tile_pool(name="const", bufs=1) as cpool,
            tc.tile_pool(name="acts", bufs=1) as apool,
            tc.tile_pool(name="xs", bufs=2) as xpool,
            tc.tile_pool(name="gsp", bufs=2) as gsp,
            tc.tile_pool(name="gsp3", bufs=2) as gsp3,
            tc.tile_pool(name="ex", bufs=10) as epool,
            tc.tile_pool(name="st", bufs=2) as stage,
            tc.tile_pool(name="pj", bufs=3, space="PSUM") as pjp,
            tc.tile_pool(name="pa", bufs=2, space="PSUM") as pap,
            tc.tile_pool(name="pc", bufs=3, space="PSUM") as pcp,
            tc.tile_pool(name="dram", bufs=1, space="DRAM") as dram,
        ):
            # ---------- initial loads, ordered for fastest compute start ----
            xt_tiles = {}

            def load_xt_chunk(t, hc):
                xt = xt_tiles.get(t)
                if xt is None:
                    xt = xpool.tile([128, HT * TB], bf16, tag="xt",
                                    name=f"xt{t}")
                    xt_tiles[t] = xt
                nc.sync.dma_start(
                    out=xt[:, hc * 4 * TB:(hc + 1) * 4 * TB].rearrange(
                        "p (hh n) -> p hh n", n=TB),
                    in_=xT[hc * 512:(hc + 1) * 512,
                           t * TB:(t + 1) * TB].rearrange(
                        "(hh p) n -> p hh n", p=128),
                )

            def load_xt(t):
                for hc in range(4):
                    load_xt_chunk(t, hc)

            # dummy collective first: pays the CC-channel init + cross-core
            # launch-skew barrier during the initial weight DMAs instead of
            # delaying the first real gather.
            dumb_in = dram.tile([128, 8], bf16, tag="dmb_i", name="dumb_in")
            dumb_out = dram.tile([HLOC * 128, 8], bf16, tag="dmb_o",
                                 name="dumb_out")
            nc.sync.dma_start(out=dumb_in[:], in_=pswap[:, 0:8])
            nc.gpsimd.collective_compute(
                "AllGather",
                mybir.AluOpType.bypass,
                replica_groups=GROUPS,
                ins=[dumb_in.opt()],
                outs=[dumb_out.opt()],
            )

            # pass 1 ([k, v]) needs xt0 chunk 0 + wk/wv chunk 0 first
            wk_s = cpool.tile([128, HT * D], bf16, tag="wk")
            wv_s = cpool.tile([128, HT * D], bf16, tag="wv")
            for hc in range(4):
                load_xt_chunk(0, hc)
                for w_s, w_d in ((wk_s, wk), (wv_s, wv)):
                    nc.sync.dma_start(
                        out=w_s[:, hc * 4 * D:(hc + 1) * 4 * D].rearrange(
                            "p (hh n) -> p hh n", n=D),
                        in_=w_d[hc * 512:(hc + 1) * 512, :].rearrange(
                            "(hh p) n -> p hh n", p=128),
                    )
            wq_s = cpool.tile([128, HT * HLOC * D], bf16, tag="wq")
            for hc in range(4):
                nc.sync.dma_start(
                    out=wq_s[:, hc * 4 * HLOC * D:(hc + 1) * 4 * HLOC * D]
                        .rearrange("p (hh n) -> p hh n", n=HLOC * D),
                    in_=wq[hc * 512:(hc + 1) * 512, :].rearrange(
                        "(hh p) n -> p hh n", p=128),
                )
            ps_s = cpool.tile([128, 128], bf16, tag="ps")
            nc.sync.dma_start(out=ps_s[:], in_=pswap[:])
            id_s = cpool.tile([128, 128], bf16, tag="id")
            nc.sync.dma_start(out=id_s[:], in_=ident[:])
            ones_s = cpool.tile([128, 128], bf16, tag="ones")
            nc.sync.dma_start(out=ones_s[:], in_=ones_in[:])
            cos_s = cpool.tile([D, S], bf16, tag="cos")
            nc.sync.dma_start(out=cos_s[:], in_=cosE[:])
            sin_s = cpool.tile([D, S], bf16, tag="sin")
            nc.sync.dma_start(out=sin_s[:], in_=sinP[:])
            mm_s = None
            if n_u:
                mm_s = cpool.tile([128, n_u * TB], bf16, tag="mm")
                nc.sync.dma_start(
                    out=mm_s[:].rearrange("p (u n) -> p u n", n=TB),
                    in_=mmask[:].rearrange("(u p) n -> p u n", p=128),
                )

            # persistent activations
            qT_s = apool.tile([128, HLOC * S], bf16, tag="qT")
            kT_s = apool.tile([128, S], bf16, tag="kT")
            v_s = apool.tile([128, S], bf16, tag="v")
            ctxT_s = apool.tile([128, HLOC * S], bf16, tag="ctxT")
            wo_s = apool.tile([128, HT * TB], bf16, tag="wo")

            # bounce/gather buffers: qb 0..2 full (4 heads); qb 3 per head
            bounce = [dram.tile([128, HLOC * TB], bf16, tag=f"bn{qb}",
                                name=f"bounce{qb}") for qb in range(3)]
            bounce3 = [dram.tile([128, 2 * TB], bf16, tag=f"bn3{i}",
                                 name=f"bounce3{i}") for i in range(2)]
            gath = [dram.tile([HLOC * 128, HLOC * TB], bf16, tag=f"g{qb}",
                              name=f"gath{qb}") for qb in range(3)]
            gath3 = [dram.tile([HLOC * 128, 2 * TB], bf16, tag=f"g3{i}",
                               name=f"gath3{i}") for i in range(2)]
            gs_tiles = {}

            def emit_cc(qb, half=None):
                if half is None:
                    bc, gt = bounce[qb], gath[qb]
                    n = HLOC * TB
                    key = qb
                    tag = "gs"
                else:
                    bc, gt = bounce3[half], gath3[half]
                    n = 2 * TB
                    key = (3, half)
                    tag = "gs3"
                nc.gpsimd.collective_compute(
                    "AllGather",
                    mybir.AluOpType.bypass,
                    replica_groups=GROUPS,
                    ins=[bc.opt()],
                    outs=[gt.opt()],
                )
                pool = gsp if half is None else gsp3
                gs = pool.tile([128, HLOC * n], bf16, tag=tag,
                               name=f"gs{key}")
                nc.sync.dma_start(
                    out=gs[:].rearrange("p (j n) -> p j n", n=n),
                    in_=gt[:].rearrange("(j p) n -> p j n", p=128),
                )
                gs_tiles[key] = gs

            # ---------------- building blocks ----------------
            def emit_proj(t):
                """QKV projection + RoPE + V transpose for token block t."""
                xt = xt_tiles[t]
                if t + 1 < NTB:
                    load_xt(t + 1)

                def w_of(kind, i, h):
                    if kind == "q":
                        return wq_s[:, h * HLOC * D + i * D:
                                    h * HLOC * D + (i + 1) * D]
                    if kind == "k":
                        return wk_s[:, h * D:(h + 1) * D]
                    return wv_s[:, h * D:(h + 1) * D]

                def rope(ch):
                    sw = pap.tile([128, TB], f32, tag="aux", name="sw")
                    mm(sw[:], ps_s[:], ch, start=True, stop=True)
                    swm = stage.tile([128, TB], bf16, tag="swm")
                    nc.vector.tensor_mul(swm[:], sw[:],
                                         sin_s[:, t * TB:(t + 1) * TB])
                    nc.vector.tensor_mul(ch, ch, cos_s[:, t * TB:(t + 1) * TB])
                    nc.vector.tensor_add(ch, ch, swm[:])

                passes = [[("k", 0), ("v", 0)], [("q", 0), ("q", 1)],
                          [("q", 2), ("q", 3)]]
                vstg = None
                pending_rope = []
                for pi, pa_ in enumerate(passes):
                    ptiles = []
                    for kind, i in pa_:
                        pt = pjp.tile([128, TB], f32, tag="pj",
                                      name=f"pj_{t}_{kind}{i}")
                        ptiles.append((pt, kind, i))
                    for h in range(HT):
                        xs = xt[:, h * TB:(h + 1) * TB]
                        st_, sp_ = (h == 0), (h == HT - 1)
                        for pt, kind, i in ptiles:
                            mm(pt[:], w_of(kind, i, h), xs,
                               start=st_, stop=sp_)
                    # drain + rope for this pass
                    for pt, kind, i in ptiles:
                        if kind == "q":
                            sl = qT_s[:, i * S + t * TB: i * S + (t + 1) * TB]
                            nc.scalar.copy(sl, pt[:])
                            pending_rope.append(sl)
                        elif kind == "k":
                            sl = kT_s[:, t * TB:(t + 1) * TB]
                            nc.scalar.copy(sl, pt[:])
                            pending_rope.append(sl)
                        else:
                            vstg = stage.tile([128, TB], bf16, tag="vstg")
                            nc.vector.tensor_copy(vstg[:], pt[:])
                    if pi == 1:
                        # k rope sw-mm + V transposes: their DVE inputs were
                        # produced during pass 1, so no PE stall here.
                        for ch in pending_rope:
                            rope(ch)
                        pending_rope = []
                        for i in range(TB // 128):
                            tps = pap.tile([128, 128], bf16, tag="aux",
                                           name="tps")
                            nc.tensor.transpose(
                                tps[:], vstg[:, i * 128:(i + 1) * 128],
                                id_s[:])
                            tt = t * (TB // 128) + i
                            nc.vector.tensor_copy(
                                v_s[:, tt * 128:(tt + 1) * 128], tps[:])
                for ch in pending_rope:
                    rope(ch)

            def emit_attn(qb, filler=None, fill_start=0):
                """Attention for all 4 local heads on q block qb.

                filler: optional list of zero-arg callables (each emits one
                small chunk of independent PE work, e.g. an out-proj matmul)
                woven between attention matmuls to keep the PE busy while
                the Scalar engine works through the exps.

                Denominator: adjacent exp tiles are pair-summed on DVE, then
                a ones[128x128]-matmul per pair accumulates the column sums
                into a PSUM tile already broadcast across partitions.
                """
                lv = live[qb]
                diag = [kt for kt in lv if mixd[(qb, kt)] is not None]
                full = [kt for kt in lv if mixd[(qb, kt)] is None]
                s_order = full[:2] + diag + full[2:]
                a_order = full + diag
                s_pos = {kt: i for i, kt in enumerate(s_order)}
                L = len(lv)
                nden = (len(full) + 1) // 2 + len(diag)

                def qlo_of(kt):
                    mx = mixd[(qb, kt)]
                    return 0 if mx is None else mx[1]
                assert qlo_of(a_order[0]) == 0
                DEPTH = 2
                filler = list(filler) if filler else []
                fill_i = [0]
                total_consumes = HLOC * L

                def pump(done_consumes):
                    # spread remaining filler over remaining consume slots
                    if done_consumes < fill_start:
                        return
                    rem_slots = max(1, total_consumes - done_consumes)
                    rem_fill = len(filler) - fill_i[0]
                    n = -(-rem_fill // rem_slots)  # ceil
                    for _ in range(n):
                        if fill_i[0] < len(filler):
                            filler[fill_i[0]]()
                            fill_i[0] += 1

                done = [0]
                for h in range(HLOC):
                    qslice = qT_s[:, h * S + qb * TB: h * S + (qb + 1) * TB]
                    cps = pcp.tile([128, TB], f32, tag="acc", name=f"cps{h}")
                    den = pjp.tile([128, TB], f32, tag="pj", name=f"den{h}")
                    ex_map = {}
                    pend = []
                    den_i = [0]

                    def den_mm(arg, qlo, qn):
                        mm(den[:, qlo:qlo + qn], ones_s[:], arg,
                           start=(den_i[0] == 0),
                           stop=(den_i[0] == nden - 1),
                           skip_group_check=True)
                        den_i[0] += 1

                    def consume(kt, a_idx):
                        ex = ex_map[kt]
                        st_, sp_ = (a_idx == 0), (a_idx == L - 1)
                        mx = mixd[(qb, kt)]
                        qlo = 0 if mx is None else mx[1]
                        qn = TB - qlo
                        mm(cps[:, qlo:TB], v_s[:, kt * 128:(kt + 1) * 128],
                           ex[:, :qn], start=st_, stop=sp_,
                           skip_group_check=True)
                        if mx is None:
                            pend.append(ex)
                            if len(pend) == 2:
                                exs = epool.tile([128, TB], bf16, tag="exs")
                                nc.vector.tensor_add(exs[:], pend[0][:],
                                                     pend[1][:])
                                den_mm(exs[:], 0, TB)
                                pend.clear()
                        else:
                            den_mm(ex[:, :qn], qlo, qn)
                        done[0] += 1
                        pump(done[0])

                    ai = 0
                    for si, kt in enumerate(s_order):
                        mx = mixd[(qb, kt)]
                        qlo = 0 if mx is None else mx[1]
                        qn = TB - qlo
                        sps = pap.tile([128, TB], f32, tag="aux", name="sps")
                        mm(sps[:, :qn], kT_s[:, kt * 128:(kt + 1) * 128],
                           qslice[:, qlo:TB], start=True, stop=True)
                        ex = epool.tile([128, TB], bf16, tag="ex")
                        nc.scalar.activation(ex[:, :qn], sps[:, :qn], EXP,
                                             scale=SCALE)
                        if mx is not None:
                            u, _, mlo, mhi = mx
                            nc.vector.tensor_mul(
                                ex[:, mlo - qlo:mhi - qlo],
                                ex[:, mlo - qlo:mhi - qlo],
                                mm_s[:, u * TB:u * TB + (mhi - mlo)])
                        ex_map[kt] = ex
                        while ai < L and s_pos[a_order[ai]] <= si - DEPTH:
                            consume(a_order[ai], ai)
                            ai += 1
                    while ai < L:
                        consume(a_order[ai], ai)
                        ai += 1
                    if pend:
                        den_mm(pend[0][:], 0, TB)
                        pend.clear()
                    # normalize: reciprocal of broadcast denominator, scale
                    den_sb = stage.tile([128, TB], f32, tag="den")
                    nc.vector.reciprocal_approx_fast(den_sb[:], den[:])
                    nc.vector.tensor_mul(
                        ctxT_s[:, h * S + qb * TB: h * S + (qb + 1) * TB],
                        cps[:], den_sb[:])
                    # bounce this head's ctx chunk to DRAM right away
                    if qb < 3:
                        nc.sync.dma_start(
                            out=bounce[qb][:, h * TB:(h + 1) * TB],
                            in_=ctxT_s[:,
                                       h * S + qb * TB: h * S + (qb + 1) * TB])
                    else:
                        nc.sync.dma_start(
                            out=bounce3[h // 2][:, (h % 2) * TB:
                                                (h % 2 + 1) * TB],
                            in_=ctxT_s[:,
                                       h * S + qb * TB: h * S + (qb + 1) * TB])
                        if h == 1:
                            emit_cc(3, half=0)
                if qb < 3:
                    emit_cc(qb)
                else:
                    emit_cc(3, half=1)

            def outproj_closures(qb):
                """List of callables, each emitting one out-proj step."""
                steps = []
                state = {}

                def mk_mm(i, gi, j, h):
                    def go():
                        if "ops" not in state or state["opi"] != i:
                            state["ops"] = pcp.tile([128, TB], f32, tag="acc",
                                                    name=f"ops{qb}_{i}")
                            state["opi"] = i
                        ops = state["ops"]
                        g = 4 * j + h
                        if qb < 3:
                            gs = gs_tiles[qb]
                            lhs = gs[:, j * HLOC * TB + h * TB + i * 128:
                                     j * HLOC * TB + h * TB + (i + 1) * 128]
                        else:
                            gs = gs_tiles[(3, h // 2)]
                            lhs = gs[:, j * 2 * TB + (h % 2) * TB + i * 128:
                                     j * 2 * TB + (h % 2) * TB + (i + 1) * 128]
                        mm(ops[:], lhs, wo_s[:, g * TB:(g + 1) * TB],
                           start=(gi == 0), stop=(gi == HT - 1))
                    return go

                def mk_store(i):
                    def go():
                        ops = state["ops"]
                        osb = stage.tile([128, TB], f32, tag="osb")
                        nc.vector.tensor_copy(osb[:], ops[:])
                        tt = qb * 4 + i
                        nc.sync.dma_start(
                            out=out_o[tt * 128:(tt + 1) * 128, :],
                            in_=osb[:])
                    return go

                if qb < 3:
                    order = [(j, h) for j in range(HLOC) for h in range(HLOC)]
                else:
                    order = [(j, h) for h in range(HLOC) for j in range(HLOC)]
                for i in range(4):
                    for gi, (j, h) in enumerate(order):
                        steps.append(mk_mm(i, gi, j, h))
                    steps.append(mk_store(i))
                return steps

            def emit_outproj(qb):
                for step in outproj_closures(qb):
                    step()

            # ---------------- the interleaved schedule ----------------
            emit_proj(0)
            emit_attn(0)
            # wo load: needed first by outproj(0) in round 2
            nc.sync.dma_start(
                out=wo_s[:].rearrange("p (h n) -> p h n", n=TB),
                in_=wo[:].rearrange("(h p) n -> p h n", p=128),
            )
            emit_proj(1)
            emit_attn(1)
            emit_proj(2)
            emit_attn(2, filler=outproj_closures(0), fill_start=12)
            emit_proj(3)
            emit_attn(3, filler=outproj_closures(1))
            # outproj(3) split: heads 0-1 immediately (their gather landed
            # mid-attn), partials parked in SBUF; heads 2-3 after outproj(2)
            # so the final gather's latency is fully covered by PE work.
            partA = []
            for i in range(4):
                ops = pcp.tile([128, TB], f32, tag="acc", name=f"op3a{i}")
                order_a = [(j, h) for h in (0, 1) for j in range(HLOC)]
                for gi, (j, h) in enumerate(order_a):
                    g = 4 * j + h
                    gs = gs_tiles[(3, 0)]
                    mm(ops[:],
                       gs[:, j * 2 * TB + h * TB + i * 128:
                          j * 2 * TB + h * TB + (i + 1) * 128],
                       wo_s[:, g * TB:(g + 1) * TB],
                       start=(gi == 0), stop=(gi == len(order_a) - 1))
                pa = cpool.tile([128, TB], bf16, tag=f"opA{i}")
                nc.vector.tensor_copy(pa[:], ops[:])
                partA.append(pa)
            emit_outproj(2)
            for i in range(4):
                ops = pcp.tile([128, TB], f32, tag="acc", name=f"op3b{i}")
                order_b = [(j, h) for h in (2, 3) for j in range(HLOC)]
                for gi, (j, h) in enumerate(order_b):
                    g = 4 * j + h
                    gs = gs_tiles[(3, 1)]
                    mm(ops[:],
                       gs[:, j * 2 * TB + (h % 2) * TB + i * 128:
                          j * 2 * TB + (h % 2) * TB + (i + 1) * 128],
                       wo_s[:, g * TB:(g + 1) * TB],
                       start=(gi == 0), stop=(gi == len(order_b) - 1))
                osb = stage.tile([128, TB], f32, tag="osb")
                nc.vector.tensor_add(osb[:], ops[:], partA[i][:])
                tt = 3 * 4 + i
                nc.sync.dma_start(out=out_o[tt * 128:(tt + 1) * 128, :],
                                  in_=osb[:])
        stk0.__exit__(None, None, None)
    nc.compile()
    return nc


def kernel(x, wq, wk, wv, wo, cos, sin, mask):
    global LAST_RESULTS
    import ml_dtypes
    from concourse.bass_utils import run_bass_kernel_spmd

    bfnp = ml_dtypes.bfloat16
    x = np.asarray(x, np.float32)
    wq = np.asarray(wq, np.float32)
    wk = np.asarray(wk, np.float32)
    wv = np.asarray(wv, np.float32)
    wo = np.asarray(wo, np.float32)
    cos = np.asarray(cos, np.float32)
    sin = np.asarray(sin, np.float32)

    live, mixd, uniq = _analyze_mask(mask)
    n_u = len(uniq)
    if n_u:
        mmask = np.concatenate(
            [np.pad(t, ((0, 0), (0, TB - t.shape[1]))) for t in uniq],
            axis=0).astype(np.float32)
    else:
        mmask = np.zeros((128, TB), np.float32)

    cosE = np.repeat(cos, 2, axis=1).T
    sp = np.repeat(sin, 2, axis=1).copy()
    sp[:, 0::2] *= -1.0
    sinP = sp.T
    pswap = np.zeros((128, 128), np.float32)
    pswap[np.arange(128), np.arange(128) ^ 1] = 1.0
    ident = np.eye(128, dtype=np.float32)

    nc = _build_program(live, mixd, n_u)

    def b(a):
        return np.ascontiguousarray(np.asarray(a).astype(bfnp))

    in_maps = []
    for c in range(N_CORES):
        bb, j = c // 4, c % 4
        in_maps.append({
            "xT": b(x[bb].T),
            "wq": b(wq[:, 512 * j:512 * (j + 1)]),
            "wk": b(wk[:, 128 * j:128 * (j + 1)]),
            "wv": b(wv[:, 128 * j:128 * (j + 1)]),
            "wo": b(wo[:, 512 * j:512 * (j + 1)]),
            "cosE": b(cosE), "sinP": b(sinP), "pswap": b(pswap),
            "ident": b(ident),
            "ones_in": b(np.ones((128, 128), np.float32)),
            "mmask": b(mmask),
        })

    res = run_bass_kernel_spmd(nc, in_maps, list(range(N_CORES)))
    LAST_RESULTS = res

    out = np.empty((B, S, HID), np.float32)
    for c in range(N_CORES):
        bb, j = c // 4, c % 4
        out[bb, :, 512 * j:512 * (j + 1)] = res.results[c]["o"]
    return out
